# revision 34
# baseline (speedup 1.0000x reference)
"""DiscoNetFusion Trainium2 kernel (8 NeuronCores, SPMD).

Strategy
--------
Only ego agent i=0 of each scene contributes to the output, so per scene b we
need the L_b = record_len[b] neighbor warps nbr[b,0,j], the 4-layer 1x1-conv
attention head on z=[nbr;ego], a softmax over j, and the weighted feature sum
followed by a channel MLP.

Core k handles output rows [10k, 10k+10) of ALL scenes (8 cores x 10 rows =
80 rows).  Per core there are sum(record_len)=9 (scene, agent) units; each
unit is 1600 output pixels (padded to 1664 = 13 tiles of 128).

Ego agents (j=0 of each scene) have an exact-identity warp, so the host
ships their features directly in both channel-major and pixel-major layout
and they skip the gather/lerp/transpose path entirely.

The remaining agents are processed in PAIRS sharing tiles: one DMA gather
per pair (concatenated dup-row source, indices offset by the source length),
one set of lerp ops covering both agents ([128, 26, C] pixel-major), one PE
transpose per px tile yielding both agents' channel-major rows at once.
conv1 is split into a nbr-half and an ego-half matmul accumulating in PSUM
(the z=[nbr;ego] concat never materializes; the ego half reuses the shared
per-scene channel-major ego tile).

conv3 uses a block-diagonal stationary (1 matmul per piece per group of 3
agents); conv4 is FUSED into the s-transpose: per px tile a tiny matmul with
the hs3 tile as stationary and a block-column w4 moving operand writes
s[px, col] directly in pixel-major PSUM.  Softmax + attention then run in
pixel-major where every op is [128, 13, 9]-sized (~100-500ns).  The weighted
sum reuses the pixel-major nbr tiles, folds per scene, and is transposed
back by PE; the MLP bias rides as a 65th weight row against a ones row.
"""

import dataclasses
import os

import numpy as np

import concourse.bacc as bacc
import concourse.mybir as mybir
from concourse.bass_utils import run_bass_kernel_spmd
from concourse.tile import TileContext

F32 = mybir.dt.float32
F16 = mybir.dt.float16
I16 = mybir.dt.int16
Alu = mybir.AluOpType
Act = mybir.ActivationFunctionType

C = 64
H = 80
W = 160
B = 3
L = 4
EPS = 1e-5
NCORES = 8
R = H // NCORES            # output rows per core
PX = R * W                 # 1600 real pixels
NT = 13                    # px tiles of 128
PXP = NT * 128             # 1664 padded pixels
NENT = H * W               # gather source entries per agent
NIDX = 2 * PXP // 16       # idx columns per pair (16-wrapped)
HCHUNKS = [(0, 832, [(0, 512), (512, 320)]), (832, 832, [(0, 512), (512, 320)])]
# u transposes write 128-wide blocks; chunks must be tile-aligned
UCHUNKS = [(0, 896), (896, 768)]
MCHUNKS = [(0, 832, [(0, 512), (512, 320)]), (832, 768, [(0, 512), (512, 256)])]
# packed fp16 const block column offsets
O_T1, O_T2, O_T3 = 0, 128, 256
O_W2, O_BD3, O_BD4 = 384, 416, 544
O_MLP, O_ID, O_CMB = 548, 612, 740
# packed fp32 const block columns: 0:6 sb, 6 cb4, 7 sb2, 8 sb3
def _nc16(NA):
    return O_CMB + 2 * NA * NT


def _wrap_idx(idx_flat):
    """[N] -> [128, N//16] wrapped-in-16-partitions, replicated to 8 groups."""
    n = idx_flat.shape[0]
    w = idx_flat.reshape(n // 16, 16).T  # [16, N//16]
    return np.tile(w, (8, 1)).astype(np.int16)


def _host_warp_prep(theta, h0):
    """Per-(unit) gather indices + lerp scalars for output rows [h0,h0+R)."""
    ys = np.linspace(-1.0, 1.0, H, dtype=np.float32)[h0 : h0 + R]
    xs = np.linspace(-1.0, 1.0, W, dtype=np.float32)
    gx, gy = np.meshgrid(xs, ys)  # [R, W]
    sx = theta[0, 0] * gx + theta[0, 1] * gy + theta[0, 2]
    sy = theta[1, 0] * gx + theta[1, 1] * gy + theta[1, 2]
    px = (sx + 1.0) * (W - 1) / 2.0
    py = (sy + 1.0) * (H - 1) / 2.0
    x0 = np.floor(px).astype(np.int64)
    y0 = np.floor(py).astype(np.int64)
    fx = (px - x0).astype(np.float32)
    fy = (py - y0).astype(np.float32)

    scale = np.ones_like(fx)
    # x handling
    x0c = np.clip(x0, 0, W - 1)
    fxp = fx.copy()
    m = x0 == W - 1          # x1 out of bounds -> drop B/D taps
    fxp[m] = 0.0
    scale[m] *= 1.0 - fx[m]
    m = x0 == -1             # x0 out of bounds -> entry at x=0 is the B tap
    x0c[m] = 0
    fxp[m] = 0.0
    scale[m] *= fx[m]
    m = (x0 < -1) | (x0 > W - 1)
    x0c[m] = 0
    fxp[m] = 0.0
    scale[m] = 0.0
    # y handling (entry [y0] holds rows y0,y0+1; row 80 half is zeros)
    y0c = np.clip(y0, 0, H - 1)
    fyp = fy.copy()
    m = y0 == -1             # row0 is the F tap
    y0c[m] = 0
    fyp[m] = 0.0
    scale[m] *= fy[m]
    m = (y0 < -1) | (y0 > H - 1)
    y0c[m] = 0
    fyp[m] = 0.0
    scale[m] = 0.0

    idx = (y0c * W + x0c).reshape(-1)
    c0 = (scale * (1.0 - fyp)).reshape(-1)
    c1 = (scale * fyp).reshape(-1)
    fxp = fxp.reshape(-1)

    pad = PXP - PX
    idx = np.concatenate([idx, np.zeros(pad, np.int64)])
    fxp = np.concatenate([fxp, np.zeros(pad, np.float32)])
    c0 = np.concatenate([c0, np.zeros(pad, np.float32)])
    c1 = np.concatenate([c1, np.zeros(pad, np.float32)])
    return idx, fxp, c0, c1


def _host_warp_mask(mask_bj, theta, h0):
    """Bilinear warp of one [H,W] mask (zero padding) for rows [h0,h0+R)."""
    ys = np.linspace(-1.0, 1.0, H, dtype=np.float32)[h0 : h0 + R]
    xs = np.linspace(-1.0, 1.0, W, dtype=np.float32)
    gx, gy = np.meshgrid(xs, ys)
    sx = theta[0, 0] * gx + theta[0, 1] * gy + theta[0, 2]
    sy = theta[1, 0] * gx + theta[1, 1] * gy + theta[1, 2]
    px = (sx + 1.0) * (W - 1) / 2.0
    py = (sy + 1.0) * (H - 1) / 2.0
    x0 = np.floor(px).astype(np.int64)
    y0 = np.floor(py).astype(np.int64)
    wx = (px - x0).astype(np.float32)
    wy = (py - y0).astype(np.float32)

    def gat(xi, yi):
        inb = ((xi >= 0) & (xi < W) & (yi >= 0) & (yi < H)).astype(np.float32)
        v = mask_bj[np.clip(yi, 0, H - 1), np.clip(xi, 0, W - 1)]
        return v * inb

    out = (
        gat(x0, y0) * (1 - wx) * (1 - wy)
        + gat(x0 + 1, y0) * wx * (1 - wy)
        + gat(x0, y0 + 1) * (1 - wx) * wy
        + gat(x0 + 1, y0 + 1) * wx * wy
    )
    return out.reshape(-1)  # [PX]


def _layout(scene_of):
    """Scene starts/counts, ego set, non-ego pairs, conv groups, col perm."""
    nb = max(scene_of) + 1
    start = [None] * nb
    cnt = [0] * nb
    for a, b in enumerate(scene_of):
        if start[b] is None:
            start[b] = a
        cnt[b] += 1
    egos = [start[b] for b in range(nb)]
    non_ego = [j for j in range(len(scene_of)) if j not in egos]
    pairs = [(j,) for j in non_ego]
    # conv2..4 groups are scene-aligned (chunks of <=3 agents; matmul psum
    # writes must start at partition 0/32/64) so each scene's attention can
    # start as soon as its own convs finish
    groups = []
    for b in range(nb):
        ag = list(range(start[b], start[b] + cnt[b]))
        groups.extend(ag[i : i + 3] for i in range(0, len(ag), 3))
    col_of = {j: j for j in range(len(scene_of))}
    return start, cnt, egos, pairs, groups, col_of


def _runs(cols):
    """Split a sorted int list into (start, len) runs of consecutive ints."""
    runs = []
    for c in cols:
        if runs and c == runs[-1][0] + runs[-1][1]:
            runs[-1] = (runs[-1][0], runs[-1][1] + 1)
        else:
            runs.append((c, 1))
    return runs


def _ap(v, offset, dims):
    """Replace the free dims of AP v (keeping partition dim)."""
    return dataclasses.replace(
        v, offset=v.offset + offset, ap=[list(v.ap[0])] + [list(d) for d in dims])


def _build_program(nagents, scene_of, src_names):
    """Build the SPMD Bass program (identical for all cores)."""
    nc = bacc.Bacc("TRN2", target_bir_lowering=False, num_devices=NCORES,
                   dynamic_dma_scratch_size=16384)
    NA = nagents
    sstart, scnt, egos, pairs, groups, col_of = _layout(scene_of)
    npairs = len(pairs)
    src_of = {j: i for i, j in enumerate(jj for pr in pairs for jj in pr)}

    psrc = [
        nc.dram_tensor(nm, [NENT + 1, 2 * C], F16, kind="ExternalInput")
        for nm in src_names
    ]
    idx_all = nc.dram_tensor("idx_all", [128, npairs * NIDX], I16,
                             kind="ExternalInput")
    scal_all = nc.dram_tensor("scal_all", [128, npairs * NIDX], F16,
                              kind="ExternalInput")
    ego_all = nc.dram_tensor("ego_all", [C, B * PXP], F16, kind="ExternalInput")
    ego_pmd = nc.dram_tensor("ego_pm", [128, B * NT * C], F16,
                             kind="ExternalInput")
    cf16d = nc.dram_tensor("cf16", [128, _nc16(NA)], F16,
                           kind="ExternalInput")
    cf32d = nc.dram_tensor("cf32", [128, 16], F32, kind="ExternalInput")
    out = nc.dram_tensor("out", [B * C, PX], F32, kind="ExternalOutput")

    with TileContext(nc) as tc:
        with (
            tc.tile_pool(name="const", bufs=1) as cpool,
            tc.tile_pool(name="zs", bufs=1) as zpool,
            tc.tile_pool(name="work", bufs=2) as wpool,
            tc.tile_pool(name="att", bufs=1) as apool,
            tc.tile_pool(name="pmm", bufs=1, space="PSUM") as pmm,
            tc.tile_pool(name="ptr", bufs=2, space="PSUM") as ptr,
        ):
            # ---- constants (consolidated, ordered by first use) ----
            idx_t = cpool.tile([128, npairs * NIDX], I16)
            nc.sync.dma_start(idx_t[:], idx_all[:, :])
            ego_t = cpool.tile([C, B * PXP], F16)
            nc.sync.dma_start(ego_t[:], ego_all[:, :])
            cf16 = cpool.tile([128, _nc16(NA)], F16)
            nc.sync.dma_start(cf16[:], cf16d[:, :])
            cf32 = cpool.tile([128, 16], F32)
            nc.sync.dma_start(cf32[:], cf32d[:, :])
            scal_t = cpool.tile([128, npairs * NIDX], F16)
            nc.sync.dma_start(scal_t[:], scal_all[:, :])
            ego_pm = cpool.tile([128, B * NT, C], F16)
            nc.sync.dma_start(ego_pm[:], ego_pmd[:, :].rearrange(
                "p (t c) -> p t c", c=C))

            # channel-major pair z tiles (rows = a*64+c), px-major nbr tiles
            zp_all = [zpool.tile([128, PXP], F16, name=f"zp{p}", tag=f"zp{p}")
                      for p in range(npairs)]
            nbrp_all = [zpool.tile([128, 2 * NT, C], F16, name=f"nbp{p}",
                                   tag=f"nbp{p}")
                        for p in range(npairs)]
            h1_all = {}
            # late agents'/groups' conv evacs go to DVE (its lerp work has
            # drained by then, while Act is still saturated)
            ne_flat = [j for pr in pairs for j in pr]
            dve_evac = set(ne_flat[len(ne_flat) // 2 :])
            dve_evac_g = set(range(1, len(groups)))
            # s (pixel-major) accumulates from the fused conv4+transpose mms
            s_ps = pmm.tile([128, NT, 16], F32, tag="s_ps", bufs=1)

            def evac_relu(dst, psrc_ap, bias_ap, on_dve):
                if on_dve:
                    nc.vector.tensor_scalar(dst, psrc_ap, bias_ap, 0.0,
                                            Alu.add, Alu.max)
                else:
                    nc.scalar.activation(dst, psrc_ap, Act.Relu,
                                         bias=bias_ap, scale=1.0)

            def conv1_ego(j):
                b = scene_of[j]
                h1_j = wpool.tile([128, PXP], F16, name=f"h1_{j}",
                                  tag=f"h1_{j}", bufs=1)
                h1_all[j] = h1_j
                for (o, n, mms) in HCHUNKS:
                    p1 = pmm.tile([128, 832], F32, tag="p34", bufs=2)
                    for (mo, mn) in mms:
                        nc.tensor.matmul(
                            p1[:, mo : mo + mn], cf16[0:C, O_T3 : O_T3 + 128],
                            ego_t[:, b * PXP + o + mo : b * PXP + o + mo + mn],
                            start=True, stop=True)
                    evac_relu(h1_j[:, o : o + n], p1[:, 0:n], cf32[:, 1:2],
                              j in dve_evac)

            def conv1_pair(j, p, a):
                b = scene_of[j]
                h1_j = wpool.tile([128, PXP], F16, name=f"h1_{j}",
                                  tag=f"h1_{j}", bufs=1)
                h1_all[j] = h1_j
                zp = zp_all[p]
                for (o, n, mms) in HCHUNKS:
                    p1 = pmm.tile([128, 832], F32, tag="p34", bufs=2)
                    for (mo, mn) in mms:
                        nc.tensor.matmul(
                            p1[:, mo : mo + mn],
                            cf16[C * a : C * a + C, O_T1 : O_T1 + 128],
                            zp[C * a : C * a + C, o + mo : o + mo + mn],
                            start=True, stop=False)
                        nc.tensor.matmul(
                            p1[:, mo : mo + mn], cf16[0:C, O_T2 : O_T2 + 128],
                            ego_t[:, b * PXP + o + mo : b * PXP + o + mo + mn],
                            start=False, stop=True)
                    evac_relu(h1_j[:, o : o + n], p1[:, 0:n], cf32[:, 1:2],
                              j in dve_evac)

            def conv234(g):
                grp = groups[g]
                ng = len(grp)
                hs2 = wpool.tile([96, PXP], F16, tag="hs2", bufs=1)
                hs3 = wpool.tile([96, PXP], F16, tag="hs3", bufs=1)
                for (o, n, mms) in HCHUNKS:
                    sl = slice(o, o + n)
                    ph2 = pmm.tile([128, 832], F32, tag="p34", bufs=2)
                    for q, jj in enumerate(grp):
                        for (mo, mn) in mms:
                            nc.tensor.matmul(
                                ph2[32 * q : 32 * q + 32, mo : mo + mn],
                                cf16[:, O_W2 : O_W2 + 32],
                                h1_all[jj][:, o + mo : o + mo + mn],
                                start=True, stop=True)
                    evac_relu(hs2[0 : 32 * ng, sl], ph2[0 : 32 * ng, 0:n],
                              cf32[0 : 32 * ng, 7:8], g in dve_evac_g)
                    p34 = pmm.tile([128, 832], F32, tag="p34", bufs=2)
                    for (mo, mn) in mms:
                        nc.tensor.matmul(
                            p34[0 : 32 * ng, mo : mo + mn],
                            cf16[0 : 32 * ng, O_BD3 : O_BD3 + 32 * ng],
                            hs2[0 : 32 * ng, o + mo : o + mo + mn],
                            start=True, stop=True)
                    evac_relu(hs3[0 : 32 * ng, sl], p34[0 : 32 * ng, 0:n],
                              cf32[0 : 32 * ng, 8:9], g in dve_evac_g)
                # conv4 fused with the s transpose: per px tile,
                # s_pm[px, s0+q] = sum_c w4[c] * h3_q[32q+c, px]
                s0 = groups[g][0]
                for t in range(NT):
                    nc.tensor.matmul(
                        s_ps[:, t, s0 : s0 + ng],
                        hs3[0 : 32 * ng, 128 * t : 128 * (t + 1)],
                        cf16[0 : 32 * ng, O_BD4 : O_BD4 + ng],
                        start=True, stop=True)

            # ---- per-scene pipeline ----
            nbr_slice = {}   # agent j -> px-major nbr AP (non-ego)

            def do_pair(p, pr):
                na2 = len(pr)
                nblk = na2 * NT
                # gather: per-agent source windows, chunked to stay under
                # the SWDGE descriptor ring size
                g_t = wpool.tile([128, nblk, 4 * C], F16, tag="g", bufs=2)
                for a, j in enumerate(pr):
                    src_flat = psrc[src_of[j]][:, :].rearrange("a b -> (a b)")
                    src_win = dataclasses.replace(
                        src_flat, ap=[[2 * C, NENT], [1, 4 * C]]
                    )
                    for (t0_, tn_) in ((0, 7), (7, 6)):
                        b0 = a * NT + t0_
                        nc.gpsimd.dma_gather(
                            g_t[:, b0 : b0 + tn_, :],
                            src_win,
                            idx_t[:, p * NIDX + b0 * 8 :
                                  p * NIDX + (b0 + tn_) * 8],
                            num_idxs=tn_ * 128,
                            num_idxs_reg=tn_ * 128,
                            elem_size=4 * C,
                            elem_step=2 * C,
                            single_packet=False,
                        )
                # bilinear combine: nbr = w00*A+w10*C + w01*B+w11*D
                t1_t = wpool.tile([128, nblk, 2 * C], F16, tag="t1", bufs=2)
                t2_t = wpool.tile([128, nblk, 2 * C], F16, tag="t2", bufs=2)
                nbr_t = nbrp_all[p]
                wq = scal_t[:, p * NIDX : (p + 1) * NIDX]
                for q, dst in ((0, t1_t[:, :, 0:C]), (1, t1_t[:, :, C : 2 * C]),
                               (2, t2_t[:, :, 0:C]), (3, t2_t[:, :, C : 2 * C])):
                    w_ap = dataclasses.replace(
                        wq, offset=wq.offset + 2 * q,
                        ap=[list(wq.ap[0]), [8, nblk], [0, C // 2], [1, 2]])
                    src = g_t[:, :, q * C : (q + 1) * C]
                    nc.vector.tensor_tensor(
                        dst.rearrange("p a (c d) -> p a c d", d=2),
                        src.rearrange("p a (c d) -> p a c d", d=2),
                        w_ap, Alu.mult)
                nc.vector.tensor_tensor(t1_t[:, :, 0:C], t1_t[:, :, 0:C],
                                        t2_t[:, :, 0:C], Alu.add)
                nc.vector.tensor_tensor(t1_t[:, :, C : 2 * C],
                                        t1_t[:, :, C : 2 * C],
                                        t2_t[:, :, C : 2 * C], Alu.add)
                nc.vector.tensor_tensor(
                    nbr_t[:, 0:nblk, :], t1_t[:, :, 0:C],
                    t1_t[:, :, C : 2 * C], Alu.add)
                # transpose px-major -> channel-major into zpair
                zp = zp_all[p]
                nv = nbr_t[:]
                for t0 in range(0, NT, 7):
                    tn = min(7, NT - t0)
                    for a in range(na2):
                        tr_ps = ptr.tile([64, 896], F16, tag="tr")
                        for t in range(t0, t0 + tn):
                            src_t = _ap(nv, (a * NT + t) * C, [[1, C]])
                            nc.tensor.transpose(
                                tr_ps[:, 128 * (t - t0) : 128 * (t - t0 + 1)],
                                src_t, cf16[:, O_ID : O_ID + 128])
                        nc.scalar.activation(
                            zp[64 * a : 64 * a + 64,
                               128 * t0 : 128 * (t0 + tn)],
                            tr_ps[:, 0 : 128 * tn], Act.Copy)
                for a, j in enumerate(pr):
                    nbr_slice[j] = _ap(nbrp_all[p][:], a * NT * C,
                                       [[C, NT], [1, C]])
                    conv1_pair(j, p, a)

            def attn_scene(b):
                s0, nb = sstart[b], scnt[b]
                # e = exp(relu(s_raw + cb4)) = max(exp(s_raw + cb4), 1)
                e_b = apool.tile([128, NT, nb], F16, name=f"e{b}")
                nc.scalar.activation(e_b[:], s_ps[:, :, s0 : s0 + nb],
                                     Act.Exp, bias=cf32[:, 6:7], scale=1.0)
                nc.vector.tensor_scalar_max(e_b[:], e_b[:], 1.0)
                # ep = e * (cm != 0); al = e * cm
                ep_b = apool.tile([128, NT, nb], F16, name=f"ep{b}")
                cmz_ap = _ap(cf16[:], O_CMB + NA + s0, [[2 * NA, NT], [1, nb]])
                nc.vector.tensor_tensor(ep_b[:], e_b[:], cmz_ap, Alu.mult)
                al_b = apool.tile([128, NT, nb], F16, name=f"al{b}")
                cm_ap = _ap(cf16[:], O_CMB + s0, [[2 * NA, NT], [1, nb]])
                nc.vector.tensor_tensor(al_b[:], e_b[:], cm_ap, Alu.mult)
                # den = sum over the scene's agents; alpha = al / den
                den_b = apool.tile([128, NT, 1], F16, name=f"den{b}")
                nc.vector.tensor_tensor(den_b[:], ep_b[:, :, 0:1],
                                        ep_b[:, :, 1:2], Alu.add)
                for ck in range(2, nb):
                    nc.vector.tensor_tensor(den_b[:], den_b[:],
                                            ep_b[:, :, ck : ck + 1], Alu.add)
                rec_b = apool.tile([128, NT, 1], F16, name=f"rec{b}")
                with nc.allow_low_precision(reason="den>=1, fp16 rec ok"):
                    nc.vector.reciprocal(rec_b[:], den_b[:])
                alp_b = apool.tile([128, NT, nb], F16, name=f"alp{b}")
                r_ap = _ap(rec_b[:], 0, [[1, NT], [0, nb]])
                nc.vector.tensor_tensor(alp_b[:], al_b[:], r_ap, Alu.mult)
                # scaled_j = alpha_j * nbr_j (pixel-major), fold over agents
                u_b = apool.tile([128, NT, C], F16, name=f"upm{b}")
                scl = []
                for i in range(nb):
                    j = s0 + i
                    sc_t = apool.tile([128, NT, C], F16, name=f"scl{j}")
                    a_ap = _ap(alp_b[:], i, [[nb, NT], [0, C]])
                    srcv = (ego_pm[:, b * NT : (b + 1) * NT, :]
                            if j == egos[b] else nbr_slice[j])
                    nc.vector.tensor_tensor(sc_t[:], srcv, a_ap, Alu.mult)
                    scl.append(sc_t)
                nc.vector.tensor_tensor(u_b[:], scl[0][:], scl[1][:], Alu.add)
                for sc_t in scl[2:]:
                    nc.vector.tensor_tensor(u_b[:], u_b[:], sc_t[:], Alu.add)
                # transpose u back to channel-major, MLP (bias as 65th row
                # against the ones row), write out
                u_sb = apool.tile([C + 1, PXP], F16, name=f"usb{b}")
                nc.gpsimd.memset(u_sb[C : C + 1, :], 1.0)
                for (o, n) in UCHUNKS:
                    u_ps = ptr.tile([C, 896], F16, tag="tr")
                    for t in range(o // 128, (o + n) // 128):
                        nc.tensor.transpose(
                            u_ps[:, 128 * t - o : 128 * (t + 1) - o],
                            u_b[:, t, :], cf16[:, O_ID : O_ID + 128])
                    nc.vector.tensor_scalar(u_sb[0:C, o : o + n],
                                            u_ps[:, 0:n], 0.0, None, Alu.add)
                for (o, n, mms) in MCHUNKS:
                    mps = pmm.tile([C, 832], F32, tag="p34", bufs=2)
                    for (mo, mn) in mms:
                        nc.tensor.matmul(mps[:, mo : mo + mn],
                                         cf16[0 : C + 1, O_MLP : O_MLP + C],
                                         u_sb[:, o + mo : o + mo + mn],
                                         start=True, stop=True)
                    ob = wpool.tile([C, 832], F32, tag="ob")
                    nc.scalar.activation(ob[:, 0:n], mps[:, 0:n], Act.Copy)
                    nc.sync.dma_start(out[b * C : (b + 1) * C, o : o + n],
                                      ob[:, 0:n])

            # egos' conv1 runs first (only needs the early ego/weight DMAs),
            # then each scene: its warps + conv1s, convs, attention, output
            for j in egos:
                conv1_ego(j)
            pair_of_scene = [[(p, pr) for p, pr in enumerate(pairs)
                              if scene_of[pr[0]] == b] for b in range(B)]
            scene_groups = [[g for g in range(len(groups))
                             if scene_of[groups[g][0]] == b] for b in range(B)]
            for b in range(B):
                for (p, pr) in pair_of_scene[b]:
                    do_pair(p, pr)
                for g in scene_groups[b]:
                    conv234(g)
                attn_scene(b)

    nc.compile()
    return nc


_PROG_CACHE = {}
_LAST_RES = None


def kernel(**inputs):
    x = np.asarray(inputs["x"], np.float32)
    mask = np.asarray(inputs["mask"], np.float32)
    record_len = np.asarray(inputs["record_len"])
    ptm = np.asarray(inputs["pairwise_t_matrix"], np.float32)
    rec = [int(v) for v in record_len]
    agents = [(b, j) for b in range(B) for j in range(rec[b])]
    nagents = len(agents)
    scene_of = [b for (b, j) in agents]
    NA = nagents
    sstart, scnt, egos, pairs, groups, col_of = _layout(scene_of)
    npairs = len(pairs)

    # ---- regroup x into per-scene node features ----
    node = np.zeros((B, L, C, H, W), np.float32)
    idx0 = 0
    for b, n in enumerate(rec):
        node[b, :n] = x[idx0 : idx0 + n]
        idx0 += n

    # ---- gather sources (pairs): dup-row pixel-major fp16, concatenated ----
    def agent_src(a):
        b, j = agents[a]
        feat = node[b, j]  # [C, H, W]
        ent = np.zeros((H + 1, W, 2 * C), np.float16)
        pm = feat.transpose(1, 2, 0).astype(np.float16)  # [H, W, C]
        ent[:H, :, :C] = pm
        ent[:H - 1, :, C:] = pm[1:]
        arr = np.zeros((NENT + 1, 2 * C), np.float16)
        arr[:NENT] = ent[:H].reshape(NENT, 2 * C)
        return arr

    non_ego_list = [j for pr in pairs for j in pr]
    src_names = [f"asrc{i}" for i in range(len(non_ego_list))]
    src_arrs = {src_names[i]: agent_src(j)
                for i, j in enumerate(non_ego_list)}

    # ---- per-core index/scalar/mask/ego prep ----
    per_core = []
    for k in range(NCORES):
        h0 = k * R
        idx_cols = np.zeros((128, npairs * NIDX), np.int16)
        scal_cols = np.zeros((128, npairs * NIDX), np.float16)
        cmb_arr = np.zeros((128, NT * 2 * NA), np.float16)
        ego_arr = np.zeros((C, B * PXP), np.float16)
        ego_pm_arr = np.zeros((128, B * NT * C), np.float16)
        for b in range(B):
            ego = np.zeros((C, PXP), np.float16)
            ego[:, :PX] = node[b, 0][:, h0 : h0 + R, :].reshape(C, PX)
            ego_arr[:, b * PXP : (b + 1) * PXP] = ego
            # px-major: [PXP, C] -> [NT, 128, C] -> [128, NT*C]
            epm = ego.T.reshape(NT, 128, C).transpose(1, 0, 2)
            ego_pm_arr[:, b * NT * C : (b + 1) * NT * C] = epm.reshape(
                128, NT * C)
        for p, pr in enumerate(pairs):
            for a, j in enumerate(pr):
                b, jj = agents[j]
                theta = ptm[b, jj, 0]
                idx, fxp, c0, c1 = _host_warp_prep(theta, h0)
                c0_ = p * NIDX + a * (PXP // 16)
                idx_cols[:, c0_ : c0_ + PXP // 16] = _wrap_idx(idx)
                w00 = (c0 * (1.0 - fxp)).astype(np.float16)
                w10 = (c1 * (1.0 - fxp)).astype(np.float16)
                w01 = (c0 * fxp).astype(np.float16)
                w11 = (c1 * fxp).astype(np.float16)
                sc = scal_cols[:, p * NIDX : (p + 1) * NIDX]
                for t in range(NT):
                    pxs = slice(128 * t, 128 * (t + 1))
                    blk = a * NT + t
                    for q, wv in enumerate((w00, w10, w01, w11)):
                        sc[:, 8 * blk + 2 * q] = wv[pxs]
                        sc[:, 8 * blk + 2 * q + 1] = wv[pxs]
        for a, (b, j) in enumerate(agents):
            theta = ptm[b, j, 0]
            col = col_of[a]
            wm = _host_warp_mask(mask[b, j], theta, h0)
            wmp = np.zeros(PXP, np.float32)
            wmp[:PX] = wm
            wmz = (wmp != 0).astype(np.float32)
            wmz[PX:] = 1.0  # keep den >= 1 on padded pixels
            cm_pm = wmp.reshape(NT, 128).T.astype(np.float16)   # [128, NT]
            cmz_pm = wmz.reshape(NT, 128).T.astype(np.float16)
            for t in range(NT):
                cmb_arr[:, t * 2 * NA + col] = cm_pm[:, t]
                cmb_arr[:, t * 2 * NA + NA + col] = cmz_pm[:, t]
        per_core.append((idx_cols, scal_cols, cmb_arr, ego_arr, ego_pm_arr))

    # ---- shared small tensors ----
    def gf(n):
        return np.asarray(inputs[n], np.float32)

    sb = np.zeros((128, 6), np.float32)
    sb2v = np.zeros((128, 1), np.float32)
    sb3v = np.zeros((128, 1), np.float32)
    a1 = gf("g1") / np.sqrt(gf("rv1") + EPS)
    sb[:, 1] = gf("be1") + (gf("cb1") - gf("rm1")) * a1
    a2 = gf("g2") / np.sqrt(gf("rv2") + EPS)
    b2f = gf("be2") + (gf("cb2") - gf("rm2")) * a2
    a3 = gf("g3") / np.sqrt(gf("rv3") + EPS)
    b3f = gf("be3") + (gf("cb3") - gf("rm3")) * a3
    for q in range(4):
        sb2v[32 * q : 32 * q + 32, 0] = b2f
        sb3v[32 * q : 32 * q + 8, 0] = b3f

    w1f = (gf("w1") * a1[None, :]).astype(np.float16)  # [128, 128]
    w1n = w1f[0:C]
    w1e = w1f[C : 2 * C]
    w3f = (gf("w3") * a3[None, :]).astype(np.float16)  # [32, 8]
    w4f = gf("w4").astype(np.float16)                  # [8, 1]

    cf16a = np.zeros((128, _nc16(NA)), np.float16)
    cf16a[:, O_T1 : O_T1 + 128] = np.concatenate([w1n, w1n], axis=0)
    cf16a[0:C, O_T2 : O_T2 + 128] = w1e
    cf16a[0:C, O_T3 : O_T3 + 128] = w1n + w1e
    cf16a[:, O_W2 : O_W2 + 32] = (gf("w2") * a2[None, :]).astype(np.float16)
    for q in range(4):
        cf16a[32 * q : 32 * q + 32, O_BD3 + 32 * q : O_BD3 + 32 * q + 8] = w3f
        cf16a[32 * q : 32 * q + 8, O_BD4 + q] = w4f[:, 0]
    cf16a[0:C, O_MLP : O_MLP + C] = gf("mlp_w").astype(np.float16)
    cf16a[C, O_MLP : O_MLP + C] = gf("mlp_b").astype(np.float16)
    cf16a[:, O_ID : O_ID + 128] = np.eye(128, dtype=np.float16)

    cf32a = np.zeros((128, 16), np.float32)
    cf32a[:, 0:6] = sb
    cf32a[:, 6] = gf("cb4")[0]
    cf32a[:, 7] = sb2v[:, 0]
    cf32a[:, 8] = sb3v[:, 0]

    shared = {"cf32": cf32a}
    shared.update(src_arrs)

    key = (nagents, tuple(scene_of))
    if key not in _PROG_CACHE:
        _PROG_CACHE[key] = _build_program(nagents, scene_of, src_names)
    nc = _PROG_CACHE[key]

    in_maps = []
    for k in range(NCORES):
        idx_cols, scal_cols, cmb_arr, ego_arr, ego_pm_arr = per_core[k]
        m = dict(shared)
        cf16k = cf16a.copy()
        cf16k[:, O_CMB : O_CMB + 2 * NA * NT] = cmb_arr
        m["cf16"] = cf16k
        m["idx_all"] = idx_cols
        m["scal_all"] = scal_cols
        m["ego_all"] = ego_arr
        m["ego_pm"] = ego_pm_arr
        in_maps.append(m)

    trace = bool(os.environ.get("KERNEL_TRACE"))
    res = run_bass_kernel_spmd(nc, in_maps, core_ids=list(range(NCORES)),
                               trace=trace)
    global _LAST_RES
    _LAST_RES = res

    out = np.zeros((B, C, H, W), np.float32)
    for k in range(NCORES):
        o = res.results[k]["out"]  # [B*C, PX]
        out[:, :, k * R : (k + 1) * R, :] = o.reshape(B, C, R, W)
    return out
